# revision 9
# baseline (speedup 1.0000x reference)
"""Trainium2 kernel for ContinuousFilterConvolution (SchNet CFConv).

Math: out[b,n,:] = sum_{e: seg_i[e]=n} atom_features[b, idx_j[e], :] * F(distances[b,e])
where F(d) = ssp(ssp(rbf(d) @ W1 + b1) @ W2 + b2), ssp(x) = softplus(x) - ln2.

F is a pointwise function of the scalar distance, so the kernel tabulates F on a
fine uniform grid on-device (grid built from an on-device iota, RBF + 2-layer
MLP on G grid points, softplus composed as ln(exp(x)+c) to stay inside one ACT
table set), then per edge: dma_gather(atom row, bf16) * dma_gather(filter row,
bf16) -> per-128-edge-tile selection matrix (is_equal vs iota) -> PE matmul
accumulating into a PSUM window of 128 consecutive nodes -> window rows
scatter-added (bf16) into a per-core node-window output.

Host<->device traffic is the bottleneck under the axon tunnel, so everything is
minimized: atom features ship as bf16 quarters and are AllGathered on-device
across the 4 cores of each batch; edge indices ship in their compact 16-row
wrap (2B/edge) and are replicated to 128 partitions on-device; the output is a
per-core bf16 node window (~1.7MB) instead of the full f32 node table.

Edge groups (1024 edges) are node-aligned (padded with zero-filter edges), so
each group's PSUM window [base_g, base_g+128) fully owns its nodes; window rows
beyond a group's span carry zeros and scatter-add is order-independent.
Sharding: 8 cores = 2 batches x 4 contiguous edge-quarters; host sums the
per-core windows into the full output.
"""
import sys
sys.path.insert(0, '/opt/trn_rl_repo')
import math
import numpy as np
import ml_dtypes

try:
    import jax as _jax
    _jax.config.update("jax_compilation_cache_dir", "/tmp/jax_pcc")
    _jax.config.update("jax_persistent_cache_min_entry_size_bytes", -1)
    _jax.config.update("jax_persistent_cache_min_compile_time_secs", 0.0)
except Exception:
    pass

import concourse.bacc as bacc
import concourse.mybir as mybir
from concourse import bass
from concourse.tile import TileContext
from concourse.bass_utils import run_bass_kernel_spmd

F32 = mybir.dt.float32
BF16 = mybir.dt.bfloat16
I16 = mybir.dt.int16
I32 = mybir.dt.int32
U8 = mybir.dt.uint8
AF = mybir.ActivationFunctionType
ALU = mybir.AluOpType
BF = ml_dtypes.bfloat16

B, N, E, D, NUM_RBF, CUTOFF = 2, 25000, 400000, 128, 64, 15.0
NCORES = 8
NP4 = N // 4         # atom rows shipped per core (AllGathered on device)
G = 16384            # filter table grid points
GC = 512             # grid points per table-build step
GROUP = 1024         # edges per node-aligned group (8 tiles -> 1 psum window)
CHUNK = GROUP
LN2 = float(np.log(2.0))
H = CUTOFF / G       # grid bin width

_cache = {}


def _patch_act_tables():
    """Force every ACT function onto natural_log_exp_and_others (has square,
    exp, ln, copy, identity) so the kernel needs exactly one table load."""
    import concourse.hw_specs as hw_specs
    orig = hw_specs.get_activation_tables
    if getattr(hw_specs, "_cfconv_patched", False):
        return
    def patched(module_arch):
        t = orig(module_arch)
        return {name: (fns if name == "natural_log_exp_and_others" else set())
                for name, fns in t.items()}
    hw_specs._cfconv_patched = True
    hw_specs.get_activation_tables = patched
    bacc.get_activation_tables = patched


def _wrap16(idx):
    """int array (len % 16 == 0) -> compact dma_gather layout [16, n/16]."""
    return np.ascontiguousarray(idx.astype(np.int16).reshape(-1, 16).T)


def _build_program(n_chunks, outn):
    _patch_act_tables()
    nc = bacc.Bacc("TRN2", target_bir_lowering=False, debug=False,
                   num_devices=NCORES)

    ecap = n_chunks * CHUNK
    ngroups = n_chunks
    ntiles = ecap // 128
    ew_a = ecap // 16                  # idxa cols
    ew_f = ecap // 16                  # idxf cols
    ew_o = 8 * ngroups                 # offc cols
    atoms_part = nc.dram_tensor("atoms_part", [NP4, D], BF16,
                                kind="ExternalInput")
    edata = nc.dram_tensor("edata", [16, ew_a + ew_f + ew_o], I16,
                           kind="ExternalInput")
    segrel = nc.dram_tensor("segrel", [128, ntiles], U8,
                            kind="ExternalInput")
    fparams = nc.dram_tensor("fparams", [128, 387], F32, kind="ExternalInput")
    out = nc.dram_tensor("out", [outn, D], BF16, kind="ExternalOutput")
    tbl = nc.dram_tensor("tbl", [G + 128, D], BF16)

    with TileContext(nc) as tc:
        with tc.tile_pool(name="const", bufs=1) as cpool, \
             tc.tile_pool(name="dram", bufs=1, space="DRAM") as dpool, \
             tc.tile_pool(name="tb", bufs=2) as tpool, \
             tc.tile_pool(name="tbp", bufs=1, space="PSUM") as tppool, \
             tc.tile_pool(name="mio", bufs=2) as mpool, \
             tc.tile_pool(name="sp", bufs=4) as spool, \
             tc.tile_pool(name="gp", bufs=2, space="PSUM") as gpool:

            # ---- AllGather atom quarters within each batch's 4 cores ----
            bounce = dpool.tile([NP4, D], BF16)
            atoms_full = dpool.tile([N, D], BF16)
            nc.gpsimd.dma_start(bounce[:, :], atoms_part[:, :])
            nc.gpsimd.collective_compute(
                "AllGather", ALU.bypass,
                replica_groups=[[0, 1, 2, 3], [4, 5, 6, 7]],
                ins=[bounce.opt()], outs=[atoms_full.opt()])

            # ---- constants ----
            from concourse.masks import make_identity
            ident = cpool.tile([128, 128], F32)
            make_identity(nc, ident[:, :])
            iota_i = cpool.tile([128, 128], I32)
            nc.gpsimd.iota(iota_i[:, :], pattern=[[1, 128]], base=0,
                           channel_multiplier=0)
            iota_f = cpool.tile([128, 128], F32)
            nc.scalar.copy(iota_f[:, :], iota_i[:, :])
            fp = cpool.tile([128, 387], F32)
            nc.sync.dma_start(fp[:, :], fparams[:, :])
            w2_ap = fp[:, 0:128]
            w1_ap = fp[0:64, 128:256]
            btab = fp[0:64, 256:384]
            b1_ap = fp[:, 384:385]
            b2_ap = fp[:, 385:386]
            negg_ap = fp[0:64, 386:387]
            # replicate compact 16-row index arrays to 128 partitions
            idxa_sb = cpool.tile([128, ew_a], I16)
            idxf_sb = cpool.tile([128, ew_f], I16)
            offc_sb = cpool.tile([128, ew_o], I16)
            for k in range(8):
                p = slice(16 * k, 16 * k + 16)
                nc.sync.dma_start(idxa_sb[p, :], edata[:, 0:ew_a])
                nc.sync.dma_start(idxf_sb[p, :], edata[:, ew_a:ew_a + ew_f])
                nc.sync.dma_start(offc_sb[p, :], edata[:, ew_a + ew_f:])
            segv = cpool.tile([128, ntiles], U8)
            nc.sync.dma_start(segv[:, :], segrel[:, :])
            segf = cpool.tile([128, ntiles], F32)
            nc.scalar.copy(segf[:, :], segv[:, :])
            zrow = cpool.tile([128, D], BF16)
            nc.vector.memset(zrow[:, :], 0.0)
            nc.sync.dma_start(tbl[G:G + 128, :], zrow[:, :])
            half = cpool.tile([128, 1], F32)
            nc.vector.memset(half[:, :], 0.5)

            # ---- filter-table build ([d, g]-major chain) ----
            for gt in range(G // GC):
                sq = tpool.tile([NUM_RBF, GC], F32, tag="sq")
                for c in range(GC // 128):
                    col = gt * (GC // 128) + c
                    nc.scalar.activation(sq[:, c * 128:(c + 1) * 128],
                                         iota_f[0:NUM_RBF, :], AF.Square,
                                         bias=btab[:, col:col + 1], scale=H)
                rbf = tpool.tile([NUM_RBF, GC], F32, tag="rbf")
                nc.scalar.activation(rbf[:, :], sq[:, :], AF.Exp,
                                     scale=negg_ap)
                z1 = tppool.tile([128, GC], F32, tag="z1")
                nc.tensor.matmul(z1[:, :], w1_ap, rbf[:, :],
                                 start=True, stop=True)
                e1 = tpool.tile([128, GC], F32, tag="e1")
                nc.scalar.activation(e1[:, :], z1[:, :], AF.Exp, bias=b1_ap)
                g1 = tpool.tile([128, GC], F32, tag="g1")
                nc.scalar.activation(g1[:, :], e1[:, :], AF.Ln, bias=1.0)
                z2 = tppool.tile([128, GC], F32, tag="z2")
                nc.tensor.matmul(z2[:, :], w2_ap, g1[:, :],
                                 start=True, stop=True)
                e2 = tpool.tile([128, GC], F32, tag="e2")
                nc.scalar.activation(e2[:, :], z2[:, :], AF.Exp, bias=b2_ap)
                # ln(e2 + 0.5) = ssp2 with the -ln2 shifts folded into b2
                f2 = tpool.tile([128, GC], F32, tag="f2")
                nc.scalar.activation(f2[:, :], e2[:, :], AF.Ln,
                                     bias=half[:, 0:1])
                trow = tpool.tile([128, GC], BF16, tag="trow")
                for i in range(GC // 128):
                    pt = tppool.tile([128, 128], F32, tag="pt")
                    nc.tensor.transpose(pt[:, :], f2[:, i * 128:(i + 1) * 128],
                                        ident[:, :])
                    nc.scalar.copy(trow[:, i * 128:(i + 1) * 128], pt[:, :])
                g0 = gt * GC
                nc.sync.dma_start(
                    tbl[g0:g0 + GC, :].rearrange("(f p) d -> p f d", p=128),
                    trow[:, :].rearrange("p (f d) -> p f d", d=128))

            # ---- main edge loop ----
            tpg = GROUP // 128          # tiles per group (8)
            for ck in range(n_chunks):
                c0 = ck * (CHUNK // 16)
                neigh = mpool.tile([128, tpg, D], BF16, tag="neigh")
                nc.gpsimd.dma_gather(neigh[:, :, :], atoms_full[:, :],
                                     idxa_sb[:, c0:c0 + CHUNK // 16],
                                     CHUNK, CHUNK, D)
                filt = mpool.tile([128, tpg, D], BF16, tag="filt")
                nc.gpsimd.dma_gather(filt[:, :, :], tbl[:, :],
                                     idxf_sb[:, c0:c0 + CHUNK // 16],
                                     CHUNK, CHUNK, D)
                msgs = mpool.tile([128, tpg, D], BF16, tag="msgs")
                nc.vector.tensor_tensor(
                    msgs[:, :, :].rearrange("p a b -> p (a b)"),
                    neigh[:, :, :].rearrange("p a b -> p (a b)"),
                    filt[:, :, :].rearrange("p a b -> p (a b)"),
                    ALU.mult)

                acc = gpool.tile([128, 128], F32, tag="acc")
                for t in range(tpg):
                    tcol = ck * tpg + t
                    s_t = spool.tile([128, 128], BF16, tag="sel")
                    nc.vector.tensor_scalar(
                        s_t[:, :], iota_f[:, :],
                        segf[:, tcol:tcol + 1], None,
                        op0=ALU.is_equal)
                    nc.tensor.matmul(acc[:, :], s_t[:, :],
                                     msgs[:, t, :],
                                     start=(t == 0), stop=(t == tpg - 1))
                flush = spool.tile([128, 1, 128], BF16, tag="flush")
                nc.scalar.copy(flush[:, 0, :], acc[:, :])
                nc.gpsimd.dma_scatter_add(
                    out[:, :], flush[:, :, :],
                    offc_sb[:, ck * 8:(ck + 1) * 8],
                    128, 128, D)

    nc.finalize()
    return nc


def _make_groups(seg, idx_j, qf):
    """Pack edges into node-aligned groups of GROUP edges.
    Returns padded (idxa, idxf, segrel_per_edge, bases)."""
    eq = len(seg)
    # node boundaries in this shard (seg sorted)
    bnd = np.flatnonzero(np.diff(seg)) + 1          # start idx of each new node
    starts = np.concatenate([[0], bnd, [eq]])       # run starts + end sentinel
    ia_out, if_out, sr_out, bases = [], [], [], []
    run = 0                     # index into starts
    while starts[run] < eq:
        lo = starts[run]
        base = int(seg[lo])
        # take as many complete node-runs as fit in GROUP edges
        hi_run = np.searchsorted(starts, lo + GROUP, side="right") - 1
        hi_run = max(hi_run, run + 1)               # at least one node-run
        hi = int(starts[hi_run])
        cnt = hi - lo
        assert cnt <= GROUP, f"node with degree {cnt} > {GROUP}"
        span = int(seg[hi - 1]) - base
        assert span < 128, f"group node span {span} >= 128"
        pad = GROUP - cnt
        ia_out.append(np.concatenate([idx_j[lo:hi], np.zeros(pad, np.int64)]))
        if_out.append(np.concatenate([qf[lo:hi], np.full(pad, G, np.int64)]))
        sr_out.append(np.concatenate([seg[lo:hi] - base,
                                      np.full(pad, 127, np.int64)]))
        bases.append(base)
        run = hi_run
    return (np.concatenate(ia_out), np.concatenate(if_out),
            np.concatenate(sr_out), np.array(bases, np.int64))


def kernel(atom_features, distances, idx_j, seg_i, centers, gamma,
           W1, b1, W2, b2, _trace=False):
    atom_features = np.asarray(atom_features, dtype=np.float32)
    distances = np.asarray(distances, dtype=np.float32)
    idx_j = np.asarray(idx_j).astype(np.int64)
    seg_i = np.asarray(seg_i).astype(np.int64)
    centers = np.asarray(centers, dtype=np.float32)
    gamma = np.asarray(gamma, dtype=np.float32)
    W1 = np.asarray(W1, dtype=np.float32)
    b1 = np.asarray(b1, dtype=np.float32)
    W2 = np.asarray(W2, dtype=np.float32)
    b2 = np.asarray(b2, dtype=np.float32)

    atoms_bf = atom_features.astype(BF)

    eq = E // 4
    shards = []
    max_groups = 0
    max_span = 0
    w0s = []
    for c in range(NCORES):
        b, q = c // 4, c % 4
        lo, hi = q * eq, (q + 1) * eq
        dd = distances[b, lo:hi]
        qf = np.clip(np.floor(dd / H), 0, G - 1).astype(np.int64)
        ia, if_, sr, bases = _make_groups(seg_i[lo:hi], idx_j[lo:hi], qf)
        w0 = int(seg_i[lo])
        w0s.append(w0)
        shards.append((ia, if_, sr, bases))
        max_groups = max(max_groups, len(bases))
        max_span = max(max_span, int(seg_i[hi - 1]) - w0 + 1)

    n_chunks = max_groups
    ngroups = n_chunks
    ecap = ngroups * GROUP
    outn = -(-(max_span + 127) // 128) * 128   # window rows before dump zone

    key = (n_chunks, outn)
    if key not in _cache:
        _cache[key] = _build_program(n_chunks, outn)
    nc = _cache[key]

    # fparams: [128, 387] f32, shared by all cores
    fparams = np.zeros((128, 387), np.float32)
    fparams[:, 0:128] = W2
    fparams[0:64, 128:256] = W1
    # btab[r, gt*4+c] = (gt*512 + c*128 + 0.5)*H - centers[r]
    col_base = (np.arange(128) * 128 + 0.5) * H          # per 128-col block
    fparams[0:64, 256:384] = col_base[None, :] - centers[:, None]
    fparams[:, 384] = b1
    fparams[:, 385] = (b2 - LN2 * W2.sum(axis=0)) - LN2  # fold both ssp shifts
    fparams[0:64, 386] = -gamma

    arange128 = np.arange(128, dtype=np.int64)
    in_maps = []
    for c in range(NCORES):
        b, q = c // 4, c % 4
        ia, if_, sr, bases = shards[c]
        padg = ngroups - len(bases)
        pade = ecap - len(ia)
        ia = np.concatenate([ia, np.zeros(pade, np.int64)])
        if_ = np.concatenate([if_, np.full(pade, G, np.int64)])
        sr = np.concatenate([sr, np.full(pade, 127, np.int64)])
        # pad groups scatter-add zeros into rows [0, 128): harmless
        rel = np.concatenate([bases - w0s[c], np.zeros(padg, np.int64)])
        # offc block for group g: [16, 8] with w[i, j] = rel_g + j*16 + i
        offc = _wrap16((rel[:, None] + arange128[None, :]).reshape(-1))
        edata = np.concatenate([_wrap16(ia), _wrap16(if_), offc], axis=1)
        segrel = np.ascontiguousarray(sr.reshape(-1, 128).T.astype(np.uint8))
        in_maps.append({
            "atoms_part": np.ascontiguousarray(
                atoms_bf[b, q * NP4:(q + 1) * NP4]),
            "edata": edata,
            "segrel": segrel,
            "fparams": fparams,
        })

    import time as _time
    _t0 = _time.perf_counter()
    res = run_bass_kernel_spmd(nc, in_maps, core_ids=list(range(NCORES)))
    kernel._last_wall_s = _time.perf_counter() - _t0
    out = np.zeros((B, N, D), dtype=np.float32)
    for c in range(NCORES):
        b = c // 4
        w0 = w0s[c]
        rows = min(outn, N - w0)
        out[b, w0:w0 + rows] += res.results[c]["out"][:rows].astype(np.float32)
    return out


# revision 14
# speedup vs baseline: 1.8137x; 1.8137x over previous
"""Trainium2 kernel for ContinuousFilterConvolution (SchNet CFConv).

Math: out[b,n,:] = sum_{e: seg_i[e]=n} atom_features[b, idx_j[e], :] * F(distances[b,e])
where F(d) = ssp(ssp(rbf(d) @ W1 + b1) @ W2 + b2), ssp(x) = softplus(x) - ln2.

F is a pointwise function of the scalar distance, so the kernel tabulates F on a
fine uniform grid on-device (grid built from an on-device iota, RBF + 2-layer
MLP on G grid points, softplus composed as ln(exp(x)+c) to stay inside one ACT
table set), then per edge: dma_gather(atom row, bf16) * dma_gather(filter row,
bf16) -> per-128-edge-tile selection matrix (is_equal vs iota) -> PE matmul
accumulating into a PSUM window of 128 consecutive nodes -> window rows
scatter-added (bf16) into a per-core node-window output.

Host<->device traffic is the bottleneck under the axon tunnel, so everything is
minimized: atom features ship as bf16 quarters and are AllGathered on-device
across the 4 cores of each batch; edge indices ship in their compact 16-row
wrap (2B/edge) and are replicated to 128 partitions on-device; the output is a
per-core bf16 node window (~1.7MB) instead of the full f32 node table.

Edge groups (1024 edges) are node-aligned (padded with zero-filter edges), so
each group's PSUM window [base_g, base_g+128) fully owns its nodes; window rows
beyond a group's span carry zeros and scatter-add is order-independent.
Sharding: 8 cores = 2 batches x 4 contiguous edge-quarters; host sums the
per-core windows into the full output.
"""
import sys
sys.path.insert(0, '/opt/trn_rl_repo')
import math
import numpy as np
import ml_dtypes

try:
    import jax as _jax
    _jax.config.update("jax_compilation_cache_dir", "/tmp/jax_pcc")
    _jax.config.update("jax_persistent_cache_min_entry_size_bytes", -1)
    _jax.config.update("jax_persistent_cache_min_compile_time_secs", 0.0)
except Exception:
    pass

import concourse.bacc as bacc
import concourse.mybir as mybir
from concourse import bass
from concourse.tile import TileContext
from concourse.bass_utils import run_bass_kernel_spmd

F32 = mybir.dt.float32
BF16 = mybir.dt.bfloat16
I16 = mybir.dt.int16
I32 = mybir.dt.int32
U8 = mybir.dt.uint8
I8 = mybir.dt.int8
AX = mybir.AxisListType
AF = mybir.ActivationFunctionType
ALU = mybir.AluOpType
BF = ml_dtypes.bfloat16

B, N, E, D, NUM_RBF, CUTOFF = 2, 25000, 400000, 128, 64, 15.0
NCORES = 8
NP4 = N // 4         # atom rows shipped per core (AllGathered on device)
G = 16384            # filter table grid points
GC = 512             # grid points per table-build step
GROUP = 1024         # edges per node-aligned group (8 tiles -> 1 psum window)
CHUNK = GROUP
LN2 = float(np.log(2.0))
H = CUTOFF / G       # grid bin width

_cache = {}


def _patch_act_tables():
    """Force every ACT function onto natural_log_exp_and_others (has square,
    exp, ln, copy, identity) so the kernel needs exactly one table load."""
    import concourse.hw_specs as hw_specs
    orig = hw_specs.get_activation_tables
    if getattr(hw_specs, "_cfconv_patched", False):
        return
    def patched(module_arch):
        t = orig(module_arch)
        return {name: (fns if name == "natural_log_exp_and_others" else set())
                for name, fns in t.items()}
    hw_specs._cfconv_patched = True
    hw_specs.get_activation_tables = patched
    bacc.get_activation_tables = patched


def _wrap16(idx):
    """int array (len % 16 == 0) -> compact dma_gather layout [16, n/16]."""
    return np.ascontiguousarray(idx.astype(np.int16).reshape(-1, 16).T)


def _build_program(n_chunks, outn):
    _patch_act_tables()
    nc = bacc.Bacc("TRN2", target_bir_lowering=False, debug=False,
                   num_devices=NCORES)

    ecap = n_chunks * CHUNK
    ngroups = n_chunks
    ntiles = ecap // 128
    ew_a = ecap // 16                  # idxa cols
    ew_f = ecap // 16                  # idxf cols
    ew_o = 8 * ngroups                 # offc cols
    atoms_part = nc.dram_tensor("atoms_part", [NP4, D], BF16,
                                kind="ExternalInput")
    edata = nc.dram_tensor("edata", [16, ew_a + ew_f + ew_o], I16,
                           kind="ExternalInput")
    segrel = nc.dram_tensor("segrel", [128, ntiles], U8,
                            kind="ExternalInput")
    fparams = nc.dram_tensor("fparams", [128, 387], F32, kind="ExternalInput")
    qout = nc.dram_tensor("qout", [outn, D], I8, kind="ExternalOutput")
    sout = nc.dram_tensor("sout", [128, outn // 128], F32,
                          kind="ExternalOutput")
    tbl = nc.dram_tensor("tbl", [G + 128, D], BF16)
    win = nc.dram_tensor("win", [outn, D], BF16)

    with TileContext(nc) as tc:
        with tc.tile_pool(name="const", bufs=1) as cpool, \
             tc.tile_pool(name="dram", bufs=1, space="DRAM") as dpool, \
             tc.tile_pool(name="tb", bufs=2) as tpool, \
             tc.tile_pool(name="tbp", bufs=1, space="PSUM") as tppool, \
             tc.tile_pool(name="mio", bufs=2) as mpool, \
             tc.tile_pool(name="sp", bufs=4) as spool, \
             tc.tile_pool(name="gp", bufs=2, space="PSUM") as gpool:

            # ---- AllGather atom quarters within each batch's 4 cores ----
            bounce = dpool.tile([NP4, D], BF16)
            atoms_full = dpool.tile([N, D], BF16)
            nc.gpsimd.dma_start(bounce[:, :], atoms_part[:, :])
            nc.gpsimd.collective_compute(
                "AllGather", ALU.bypass,
                replica_groups=[[0, 1, 2, 3], [4, 5, 6, 7]],
                ins=[bounce.opt()], outs=[atoms_full.opt()])

            # ---- constants ----
            from concourse.masks import make_identity
            ident = cpool.tile([128, 128], F32)
            make_identity(nc, ident[:, :])
            iota_i = cpool.tile([128, 128], I32)
            nc.gpsimd.iota(iota_i[:, :], pattern=[[1, 128]], base=0,
                           channel_multiplier=0)
            iota_f = cpool.tile([128, 128], F32)
            nc.scalar.copy(iota_f[:, :], iota_i[:, :])
            fp = cpool.tile([128, 387], F32)
            nc.sync.dma_start(fp[:, :], fparams[:, :])
            w2_ap = fp[:, 0:128]
            w1_ap = fp[0:64, 128:256]
            btab = fp[0:64, 256:384]
            b1_ap = fp[:, 384:385]
            b2_ap = fp[:, 385:386]
            negg_ap = fp[0:64, 386:387]
            # replicate compact 16-row index arrays to 128 partitions
            idxa_sb = cpool.tile([128, ew_a], I16)
            idxf_sb = cpool.tile([128, ew_f], I16)
            offc_sb = cpool.tile([128, ew_o], I16)
            for k in range(8):
                p = slice(16 * k, 16 * k + 16)
                nc.sync.dma_start(idxa_sb[p, :], edata[:, 0:ew_a])
                nc.sync.dma_start(idxf_sb[p, :], edata[:, ew_a:ew_a + ew_f])
                nc.sync.dma_start(offc_sb[p, :], edata[:, ew_a + ew_f:])
            segv = cpool.tile([128, ntiles], U8)
            nc.sync.dma_start(segv[:, :], segrel[:, :])
            segf = cpool.tile([128, ntiles], F32)
            nc.scalar.copy(segf[:, :], segv[:, :])
            zrow = cpool.tile([128, D], BF16)
            nc.vector.memset(zrow[:, :], 0.0)
            nc.sync.dma_start(tbl[G:G + 128, :], zrow[:, :])
            half = cpool.tile([128, 1], F32)
            nc.vector.memset(half[:, :], 0.5)
            # zero the accumulation window (internal dram persists per call)
            zwin = cpool.tile([128, outn], BF16)
            nc.vector.memset(zwin[:, :], 0.0)
            nc.sync.dma_start(
                win[:, :].rearrange("(f p) d -> p f d", p=128),
                zwin[:, :].rearrange("p (f d) -> p f d", d=128))

            # ---- filter-table build ([d, g]-major chain) ----
            for gt in range(G // GC):
                sq = tpool.tile([NUM_RBF, GC], F32, tag="sq")
                for c in range(GC // 128):
                    col = gt * (GC // 128) + c
                    nc.scalar.activation(sq[:, c * 128:(c + 1) * 128],
                                         iota_f[0:NUM_RBF, :], AF.Square,
                                         bias=btab[:, col:col + 1], scale=H)
                rbf = tpool.tile([NUM_RBF, GC], F32, tag="rbf")
                nc.scalar.activation(rbf[:, :], sq[:, :], AF.Exp,
                                     scale=negg_ap)
                z1 = tppool.tile([128, GC], F32, tag="z1")
                nc.tensor.matmul(z1[:, :], w1_ap, rbf[:, :],
                                 start=True, stop=True)
                e1 = tpool.tile([128, GC], F32, tag="e1")
                nc.scalar.activation(e1[:, :], z1[:, :], AF.Exp, bias=b1_ap)
                g1 = tpool.tile([128, GC], F32, tag="g1")
                nc.scalar.activation(g1[:, :], e1[:, :], AF.Ln, bias=1.0)
                z2 = tppool.tile([128, GC], F32, tag="z2")
                nc.tensor.matmul(z2[:, :], w2_ap, g1[:, :],
                                 start=True, stop=True)
                e2 = tpool.tile([128, GC], F32, tag="e2")
                nc.scalar.activation(e2[:, :], z2[:, :], AF.Exp, bias=b2_ap)
                # ln(e2 + 0.5) = ssp2 with the -ln2 shifts folded into b2
                f2 = tpool.tile([128, GC], F32, tag="f2")
                nc.scalar.activation(f2[:, :], e2[:, :], AF.Ln,
                                     bias=half[:, 0:1])
                trow = tpool.tile([128, GC], BF16, tag="trow")
                for i in range(GC // 128):
                    pt = tppool.tile([128, 128], F32, tag="pt")
                    nc.tensor.transpose(pt[:, :], f2[:, i * 128:(i + 1) * 128],
                                        ident[:, :])
                    nc.scalar.copy(trow[:, i * 128:(i + 1) * 128], pt[:, :])
                g0 = gt * GC
                nc.sync.dma_start(
                    tbl[g0:g0 + GC, :].rearrange("(f p) d -> p f d", p=128),
                    trow[:, :].rearrange("p (f d) -> p f d", d=128))

            # ---- main edge loop ----
            tpg = GROUP // 128          # tiles per group (8)
            for ck in range(n_chunks):
                c0 = ck * (CHUNK // 16)
                neigh = mpool.tile([128, tpg, D], BF16, tag="neigh")
                nc.gpsimd.dma_gather(neigh[:, :, :], atoms_full[:, :],
                                     idxa_sb[:, c0:c0 + CHUNK // 16],
                                     CHUNK, CHUNK, D)
                filt = mpool.tile([128, tpg, D], BF16, tag="filt")
                nc.gpsimd.dma_gather(filt[:, :, :], tbl[:, :],
                                     idxf_sb[:, c0:c0 + CHUNK // 16],
                                     CHUNK, CHUNK, D)
                msgs = mpool.tile([128, tpg, D], BF16, tag="msgs")
                nc.vector.tensor_tensor(
                    msgs[:, :, :].rearrange("p a b -> p (a b)"),
                    neigh[:, :, :].rearrange("p a b -> p (a b)"),
                    filt[:, :, :].rearrange("p a b -> p (a b)"),
                    ALU.mult)

                acc = gpool.tile([128, 128], F32, tag="acc")
                for t in range(tpg):
                    tcol = ck * tpg + t
                    s_t = spool.tile([128, 128], BF16, tag="sel")
                    nc.vector.tensor_scalar(
                        s_t[:, :], iota_f[:, :],
                        segf[:, tcol:tcol + 1], None,
                        op0=ALU.is_equal)
                    nc.tensor.matmul(acc[:, :], s_t[:, :],
                                     msgs[:, t, :],
                                     start=(t == 0), stop=(t == tpg - 1))
                flush = spool.tile([128, 1, 128], BF16, tag="flush")
                nc.scalar.copy(flush[:, 0, :], acc[:, :])
                nc.gpsimd.dma_scatter_add(
                    win[:, :], flush[:, :, :],
                    offc_sb[:, ck * 8:(ck + 1) * 8],
                    128, 128, D)

            # ---- final pass: per-node int8 quantization of the window ----
            scales = cpool.tile([128, outn // 128], F32)
            for t in range(outn // 128):
                wt = spool.tile([128, D], BF16, tag="wt")
                nc.sync.dma_start(wt[:, :], win[t * 128:(t + 1) * 128, :])
                nc.vector.tensor_reduce(scales[:, t:t + 1], wt[:, :],
                                        axis=AX.X, op=ALU.max,
                                        apply_absolute_value=True)
                nc.vector.tensor_scalar_max(scales[:, t:t + 1],
                                            scales[:, t:t + 1], 1e-30)
                r = spool.tile([128, 1], F32, tag="rcp")
                nc.vector.reciprocal(r[:, :], scales[:, t:t + 1])
                s = spool.tile([128, 1], F32, tag="scl")
                nc.vector.tensor_scalar_mul(s[:, :], r[:, :], 127.0)
                q = spool.tile([128, D], I8, tag="q")
                nc.vector.tensor_scalar(q[:, :], wt[:, :], s[:, 0:1], None,
                                        op0=ALU.mult)
                nc.sync.dma_start(qout[t * 128:(t + 1) * 128, :], q[:, :])
            nc.sync.dma_start(sout[:, :], scales[:, :])

    nc.finalize()
    return nc


def _make_groups(seg, idx_j, qf):
    """Pack edges into node-aligned groups of GROUP edges.
    Returns padded (idxa, idxf, segrel_per_edge, bases)."""
    eq = len(seg)
    # node boundaries in this shard (seg sorted)
    bnd = np.flatnonzero(np.diff(seg)) + 1          # start idx of each new node
    starts = np.concatenate([[0], bnd, [eq]])       # run starts + end sentinel
    ia_out, if_out, sr_out, bases = [], [], [], []
    run = 0                     # index into starts
    while starts[run] < eq:
        lo = starts[run]
        base = int(seg[lo])
        # take as many complete node-runs as fit in GROUP edges
        hi_run = np.searchsorted(starts, lo + GROUP, side="right") - 1
        hi_run = max(hi_run, run + 1)               # at least one node-run
        hi = int(starts[hi_run])
        cnt = hi - lo
        assert cnt <= GROUP, f"node with degree {cnt} > {GROUP}"
        span = int(seg[hi - 1]) - base
        assert span < 128, f"group node span {span} >= 128"
        pad = GROUP - cnt
        ia_out.append(np.concatenate([idx_j[lo:hi], np.zeros(pad, np.int64)]))
        if_out.append(np.concatenate([qf[lo:hi], np.full(pad, G, np.int64)]))
        sr_out.append(np.concatenate([seg[lo:hi] - base,
                                      np.full(pad, 127, np.int64)]))
        bases.append(base)
        run = hi_run
    return (np.concatenate(ia_out), np.concatenate(if_out),
            np.concatenate(sr_out), np.array(bases, np.int64))


def kernel(atom_features, distances, idx_j, seg_i, centers, gamma,
           W1, b1, W2, b2, _trace=False):
    atom_features = np.asarray(atom_features, dtype=np.float32)
    distances = np.asarray(distances, dtype=np.float32)
    idx_j = np.asarray(idx_j).astype(np.int64)
    seg_i = np.asarray(seg_i).astype(np.int64)
    centers = np.asarray(centers, dtype=np.float32)
    gamma = np.asarray(gamma, dtype=np.float32)
    W1 = np.asarray(W1, dtype=np.float32)
    b1 = np.asarray(b1, dtype=np.float32)
    W2 = np.asarray(W2, dtype=np.float32)
    b2 = np.asarray(b2, dtype=np.float32)

    atoms_bf = atom_features.astype(BF)

    eq = E // 4
    shards = []
    max_groups = 0
    max_span = 0
    w0s = []
    for c in range(NCORES):
        b, q = c // 4, c % 4
        lo, hi = q * eq, (q + 1) * eq
        dd = distances[b, lo:hi]
        qf = np.clip(np.floor(dd / H), 0, G - 1).astype(np.int64)
        ia, if_, sr, bases = _make_groups(seg_i[lo:hi], idx_j[lo:hi], qf)
        w0 = int(seg_i[lo])
        w0s.append(w0)
        shards.append((ia, if_, sr, bases))
        max_groups = max(max_groups, len(bases))
        max_span = max(max_span, int(seg_i[hi - 1]) - w0 + 1)

    n_chunks = max_groups
    ngroups = n_chunks
    ecap = ngroups * GROUP
    outn = -(-(max_span + 127) // 128) * 128   # window rows before dump zone

    key = (n_chunks, outn)
    if key not in _cache:
        _cache[key] = _build_program(n_chunks, outn)
    nc = _cache[key]

    # fparams: [128, 387] f32, shared by all cores
    fparams = np.zeros((128, 387), np.float32)
    fparams[:, 0:128] = W2
    fparams[0:64, 128:256] = W1
    # btab[r, gt*4+c] = (gt*512 + c*128 + 0.5)*H - centers[r]
    col_base = (np.arange(128) * 128 + 0.5) * H          # per 128-col block
    fparams[0:64, 256:384] = col_base[None, :] - centers[:, None]
    fparams[:, 384] = b1
    fparams[:, 385] = (b2 - LN2 * W2.sum(axis=0)) - LN2  # fold both ssp shifts
    fparams[0:64, 386] = -gamma

    arange128 = np.arange(128, dtype=np.int64)
    in_maps = []
    for c in range(NCORES):
        b, q = c // 4, c % 4
        ia, if_, sr, bases = shards[c]
        padg = ngroups - len(bases)
        pade = ecap - len(ia)
        ia = np.concatenate([ia, np.zeros(pade, np.int64)])
        if_ = np.concatenate([if_, np.full(pade, G, np.int64)])
        sr = np.concatenate([sr, np.full(pade, 127, np.int64)])
        # pad groups scatter-add zeros into rows [0, 128): harmless
        rel = np.concatenate([bases - w0s[c], np.zeros(padg, np.int64)])
        # offc block for group g: [16, 8] with w[i, j] = rel_g + j*16 + i
        offc = _wrap16((rel[:, None] + arange128[None, :]).reshape(-1))
        edata = np.concatenate([_wrap16(ia), _wrap16(if_), offc], axis=1)
        segrel = np.ascontiguousarray(sr.reshape(-1, 128).T.astype(np.uint8))
        in_maps.append({
            "atoms_part": np.ascontiguousarray(
                atoms_bf[b, q * NP4:(q + 1) * NP4]),
            "edata": edata,
            "segrel": segrel,
            "fparams": fparams,
        })

    import time as _time
    _t0 = _time.perf_counter()
    res = run_bass_kernel_spmd(nc, in_maps, core_ids=list(range(NCORES)))
    kernel._last_wall_s = _time.perf_counter() - _t0
    out = np.zeros((B, N, D), dtype=np.float32)
    for c in range(NCORES):
        b = c // 4
        w0 = w0s[c]
        rows = min(outn, N - w0)
        q = res.results[c]["qout"][:rows].astype(np.float32)
        sc = res.results[c]["sout"]          # [128, outn/128]; node (t,p)->[p,t]
        scale = (sc.T.reshape(-1, 1).astype(np.float32) / 127.0)[:rows]
        out[b, w0:w0 + rows] += q * scale
    return out
